# revision 8
# baseline (speedup 1.0000x reference)
"""FESTGCN Trainium2 kernel: 8-core SPMD Bass/Tile implementation (v2).

Algorithm (validated against the reference in f32/f16 numpy, sim.py):
  For t in 0..9:
    M_t = dtw * (td > 9-t) + (spec_lap + I)   [(+ lap) * 2/3 at t=9, host-baked]
    S1 += M_t @ c1_t          c1_t = [h_t | x_t]  per-b, k-order [32 h, x, pad]
    gcn1_t = 0.5 * S1 @ W1 + (t+1) b1 ;  sig_t = sigmoid(gcn1_t)
    r gating: gate(node n, ch c) = sig(node n//2, half n%2, ch c)
    S2 += M_t @ c2_t          c2_t = [r*h_t | x_t]
  u(n,c) = sig_9(node N/2 + n//2, half n%2); c = tanh(0.5 * S2 @ W2 + 10 b2)
  out = c + u*(h_9 - c)

Design vs v1 baseline:
  - M_t masks/sums/scaling precomputed on HOST, streamed per step as one
    4MB DMA ([128, 32*512] f16 strip, 32KB/partition descriptors).
  - c1 strips ([x|h] conc) precomputed on HOST in matmul-ready layout:
    one 1MB DMA per step. No on-chip transposes/casts for inputs.
  - AllGather payload pre-EXPANDED to gate-node-major rows (row 2q+h holds
    sig(q, half h)), so gate/u gathers are contiguous 256B-descriptor DMAs.
  - Software pipeline: conv2 of step t-1 runs during step t (AG hidden).
  - Epilogue: tanh-c AllGather (aux), full output mixed redundantly per core.
"""

import numpy as np

import concourse.bacc as bacc
import concourse.mybir as mybir
import concourse.tile as tile
from concourse.bass_utils import run_bass_kernel_spmd

B, T, N, H = 4, 10, 4096, 32
NC = 8
RPC = N // NC            # 512 rows per core
NT = N // 128            # 32 n-tiles
MT = RPC // 128          # 4 m-tiles per core
K = H + 2                # 34: [32 h, x, zero-pad] per-b channel order
F1 = B * K               # 136
NB = NT * B              # 128
f32 = mybir.dt.float32
f16 = mybir.dt.float16
Alu = mybir.AluOpType
Act = mybir.ActivationFunctionType
CORES = list(range(NC))


def _build_nc():
    nc = bacc.Bacc(
        "TRN2",
        target_bir_lowering=False,
        debug=False,
        enable_asserts=True,
        num_devices=NC,
    )
    mh = nc.dram_tensor("mh", [T, 128, NT * RPC], f16, kind="ExternalInput").ap()
    c1h = nc.dram_tensor("c1h", [T, 128, NT * F1], f16, kind="ExternalInput").ap()
    w1h = nc.dram_tensor("w1h", [K, 2 * H], f32, kind="ExternalInput").ap()
    w2h = nc.dram_tensor("w2h", [K, H], f32, kind="ExternalInput").ap()
    biastab = nc.dram_tensor("biastab", [T, 2 * B * H], f32,
                             kind="ExternalInput").ap()
    bias2 = nc.dram_tensor("bias2", [1, B * H], f32, kind="ExternalInput").ap()
    hout = nc.dram_tensor("hout", [128, NT * B * H], f16,
                          kind="ExternalOutput").ap()

    with tile.TileContext(nc) as tc:
        with (
            tc.tile_pool(name="mpool", bufs=3) as mpool,     # M strips (32KB/p)
            tc.tile_pool(name="c1p", bufs=3) as c1p,         # c1 strips
            tc.tile_pool(name="gp", bufs=2) as gp,           # gate/u/cl strips
            tc.tile_pool(name="c2p", bufs=2) as c2p,         # c2 strips
            tc.tile_pool(name="sm", bufs=1) as sm,           # small persistents
            tc.tile_pool(name="acc", bufs=2) as accp,        # S1/S2 ping-pong
            tc.tile_pool(name="wk", bufs=4) as wk,           # small work tiles
            tc.tile_pool(name="z1p", bufs=1, space="PSUM") as z1p,
            tc.tile_pool(name="z2p", bufs=1, space="PSUM") as z2p,
            tc.tile_pool(name="tpz", bufs=1, space="PSUM") as tpzp,
            tc.tile_pool(name="g1p", bufs=2, space="PSUM") as g1p,
            tc.tile_pool(name="dramp", bufs=1, space="DRAM") as dramp,
        ):
            agsrc = [
                dramp.tile([2 * RPC, B * H], f16, tag=f"agsrc{t}",
                           name=f"agsrc{t}")
                for t in range(T)
            ]
            agdst = [
                dramp.tile([NC * 2 * RPC, B * H], f16, tag=f"agdst{t}",
                           name=f"agdst{t}", addr_space="Shared")
                for t in range(T)
            ]
            auxsrc = dramp.tile([RPC, B * H], f16, tag="auxsrc", name="auxsrc")
            auxdst = dramp.tile([NC * RPC, B * H], f16, tag="auxdst",
                                name="auxdst", addr_space="Shared")

            # ---------------- prologue ----------------
            iota_i = wk.tile([128, 128], mybir.dt.int32, tag="iota", bufs=1)
            nc.gpsimd.iota(iota_i[:], pattern=[[1, 128]], base=0,
                           channel_multiplier=-1)
            ident = sm.tile([128, 128], f32, tag="ident")
            nc.vector.tensor_scalar(ident[:], iota_i[:], 0, None,
                                    op0=Alu.is_equal)

            w1s = sm.tile([K, 2 * H], f32, tag="w1s")
            nc.sync.dma_start(w1s[:], w1h[:])
            w2s = sm.tile([K, H], f32, tag="w2s")
            nc.sync.dma_start(w2s[:], w2h[:])
            biasf2 = sm.tile([128, B * H], f32, tag="biasf2")
            nc.sync.dma_start(biasf2[:], bias2[0:1, :].broadcast_to((128, B * H)))

            s1 = [accp.tile([128, F1], f32, tag=f"s1_{mt}", name=f"s1_{mt}")
                  for mt in range(MT)]
            s2 = [accp.tile([128, F1], f32, tag=f"s2_{mt}", name=f"s2_{mt}")
                  for mt in range(MT)]
            for mt in range(MT):
                nc.vector.memset(s1[mt][:], 0.0)
                nc.vector.memset(s2[mt][:], 0.0)

            # step-0 strips
            mtiles = {}
            c1tiles = {}
            mtiles[0] = mpool.tile([128, NT * RPC], f16, tag="m", name="m0")
            nc.sync.dma_start(mtiles[0][:], mh[0])
            c1tiles[0] = c1p.tile([128, NT * F1], f16, tag="c1", name="c10")
            nc.sync.dma_start(c1tiles[0][:], c1h[0])

            def conv(zpool, mstrip, cstrip, ztag):
                za = zpool.tile([128, 2 * F1], f32, tag=f"{ztag}a",
                                name=f"{ztag}a")
                zb = zpool.tile([128, 2 * F1], f32, tag=f"{ztag}b",
                                name=f"{ztag}b")
                zh = [za, zb]
                for mt in range(MT):
                    for nt in range(NT):
                        nc.tensor.matmul(
                            zh[mt // 2][:, (mt % 2) * F1:(mt % 2 + 1) * F1],
                            mstrip[:, nt * RPC + mt * 128:
                                   nt * RPC + (mt + 1) * 128],
                            cstrip[:, nt * F1:(nt + 1) * F1],
                            start=(nt == 0 and mt % 2 == 0),
                            stop=(nt == NT - 1),
                        )
                return zh

            def gates_for(tt):
                gstrip = gp.tile([128, NT * B * H], f16, tag="g",
                                 name=f"g{tt}")
                nc.sync.dma_start(
                    gstrip.rearrange("p (nt c) -> p nt c", c=B * H),
                    agdst[tt][0:N, :].rearrange("(nt p) c -> p nt c", p=128),
                )
                return gstrip

            def conv2_for(tt, gstrip):
                """c2 build + conv2 matmuls + S2 update."""
                c1prev = c1tiles[tt]
                c2 = c2p.tile([128, NT * F1], f16, tag="c2")
                c2v = c2.rearrange("p (nb k) -> p nb k", k=K)
                c1v = c1prev.rearrange("p (nb k) -> p nb k", k=K)
                gv = gstrip.rearrange("p (nb c) -> p nb c", c=H)
                nc.vector.tensor_mul(c2v[:, :, 0:H], gv[:], c1v[:, :, 0:H])
                nc.vector.tensor_copy(c2v[:, :, H:K], c1v[:, :, H:K])
                z2h = conv(z2p, mtiles[tt][:], c2, "z2")
                for mt in range(MT):
                    s2n = accp.tile([128, F1], f32, tag=f"s2_{mt}")
                    nc.vector.tensor_add(
                        s2n[:], s2[mt][:],
                        z2h[mt // 2][:, (mt % 2) * F1:(mt % 2 + 1) * F1],
                    )
                    s2[mt] = s2n

            # ---------------- main loop ----------------
            for t in range(T):
                # gate gather for previous step (scalar ring, waits on AG t-1)
                gcur = gates_for(t - 1) if t >= 1 else None
                # prefetch next step strips
                if t + 1 < T:
                    mtiles[t + 1] = mpool.tile([128, NT * RPC], f16, tag="m",
                                             name=f"m{t + 1}")
                    nc.sync.dma_start(mtiles[t + 1][:], mh[t + 1])
                    c1tiles[t + 1] = c1p.tile([128, NT * F1], f16, tag="c1",
                                              name=f"c1{t + 1}")
                    nc.sync.dma_start(c1tiles[t + 1][:], c1h[t + 1])

                # conv1_t
                z1h = conv(z1p, mtiles[t][:], c1tiles[t][:], "z1")

                # S1 update + small matmuls + sigmoid + AG send
                biasf = wk.tile([128, 2 * B * H], f32, tag="biasf", bufs=2,
                                name="biasf")
                nc.scalar.dma_start(
                    biasf[:], biastab[t:t + 1, :].broadcast_to((128, 2 * B * H)))
                for mt in range(MT):
                    s1n = accp.tile([128, F1], f32, tag=f"s1_{mt}")
                    nc.vector.tensor_add(
                        s1n[:], s1[mt][:],
                        z1h[mt // 2][:, (mt % 2) * F1:(mt % 2 + 1) * F1],
                    )
                    s1[mt] = s1n
                    g1 = g1p.tile([128, 2 * B * H], f32, tag="g1")
                    for b in range(B):
                        tz = tpzp.tile([K, 128], f32, tag="tz")
                        nc.tensor.transpose(
                            tz[:], s1n[:, b * K:(b + 1) * K], ident[:])
                        zbt = wk.tile([K, 128], f32, tag="zbt", bufs=6)
                        nc.scalar.copy(zbt[:], tz[:])
                        nc.tensor.matmul(
                            g1[:, b * H:(b + 1) * H],
                            zbt[:], w1s[:, 0:H], start=True, stop=True)
                        nc.tensor.matmul(
                            g1[:, B * H + b * H:B * H + (b + 1) * H],
                            zbt[:], w1s[:, H:2 * H], start=True, stop=True)
                    sigi = wk.tile([128, 2 * B * H], f32, tag="sigi")
                    nc.vector.scalar_tensor_tensor(
                        sigi[:], g1[:], 1.0, biasf[:],
                        op0=Alu.mult, op1=Alu.add)
                    sigb = wk.tile([128, 2 * B * H], f16, tag="sigb")
                    nc.scalar.activation(sigb[:], sigi[:], Act.Sigmoid)
                    nc.scalar.dma_start(
                        agsrc[t][2 * mt * 128:2 * (mt + 1) * 128, :]
                        .rearrange("(p h) c -> p h c", h=2),
                        sigb.rearrange("p (h c) -> p h c", h=2),
                    )
                nc.gpsimd.collective_compute(
                    "AllGather",
                    Alu.bypass,
                    replica_groups=[CORES],
                    ins=[agsrc[t][:]],
                    outs=[agdst[t][:]],
                )

                # conv2 for previous step (AG t-1 already landed / landing)
                if t >= 1:
                    conv2_for(t - 1, gcur)

            # ---------------- epilogue ----------------
            gfin = gates_for(T - 1)
            conv2_for(T - 1, gfin)

            # u strip prefetch (ready once AG_9 lands; overlaps tanh/aux AG)
            ustrip = gp.tile([128, NT * B * H], f16, tag="g", name="ustrip")
            nc.scalar.dma_start(
                ustrip.rearrange("p (nt c) -> p nt c", c=B * H),
                agdst[T - 1][N:2 * N, :].rearrange("(nt p) c -> p nt c", p=128),
            )

            # tanh path (own rows) -> aux AG
            for mt in range(MT):
                g2full = g1p.tile([128, 2 * B * H], f32, tag="g1", name="g2")
                g2 = g2full[:, 0:B * H]
                for b in range(B):
                    tz = tpzp.tile([K, 128], f32, tag="tz")
                    nc.tensor.transpose(
                        tz[:], s2[mt][:, b * K:(b + 1) * K], ident[:])
                    zbt = wk.tile([K, 128], f32, tag="zbt", bufs=6)
                    nc.scalar.copy(zbt[:], tz[:])
                    nc.tensor.matmul(g2[:, b * H:(b + 1) * H], zbt[:], w2s[:],
                                     start=True, stop=True)
                tani = wk.tile([128, B * H], f32, tag="tani", bufs=2)
                nc.vector.scalar_tensor_tensor(
                    tani[:], g2[:, :], 1.0, biasf2[:], op0=Alu.mult, op1=Alu.add)
                tanb = wk.tile([128, B * H], f16, tag="tanb", bufs=2)
                nc.scalar.activation(tanb[:], tani[:], Act.Tanh)
                nc.sync.dma_start(auxsrc[mt * 128:(mt + 1) * 128, :], tanb[:])
            nc.gpsimd.collective_compute(
                "AllGather",
                Alu.bypass,
                replica_groups=[CORES],
                ins=[auxsrc[:]],
                outs=[auxdst[:]],
            )

            # full-output mix (redundant on every core)
            clstrip = gp.tile([128, NT * B * H], f16, tag="g", name="clstrip")
            nc.scalar.dma_start(
                clstrip.rearrange("p (nt c) -> p nt c", c=B * H),
                auxdst[:].rearrange("(nt p) c -> p nt c", p=128),
            )
            c1v9 = c1tiles[T - 1].rearrange("p (nb k) -> p nb k", k=K)
            clv = clstrip.rearrange("p (nb c) -> p nb c", c=H)
            ds = c2p.tile([128, NT * B * H], f16, tag="ds", bufs=2, name="ds")
            nc.vector.tensor_sub(
                ds.rearrange("p (nb c) -> p nb c", c=H),
                c1v9[:, :, 0:H], clv[:])
            ms = c2p.tile([128, NT * B * H], f16, tag="ds", name="ms")
            nc.vector.tensor_mul(ms[:], ustrip[:], ds[:])
            ho = gp.tile([128, NT * B * H], f16, tag="g", name="ho")
            nc.vector.tensor_add(ho[:], ms[:], clstrip[:])
            nc.sync.dma_start(hout[:], ho[:])

    nc.finalize()
    return nc


_NC_CACHE = None


def _get_nc():
    global _NC_CACHE
    if _NC_CACHE is None:
        _NC_CACHE = _build_nc()
    return _NC_CACHE


def make_in_maps(inputs, states, dtw, spec_lap, laplacian, time_delay,
                 W1, b1, W2, b2):
    ft = np.float16
    tdc = np.ceil(np.abs(time_delay) / 1.0).astype(np.float32)
    base = spec_lap + np.eye(N, dtype=np.float32)
    baseT = np.ascontiguousarray(base.T)
    dtwT = np.ascontiguousarray(dtw.T)
    tdcT = np.ascontiguousarray(tdc.T)
    lapT = np.ascontiguousarray(laplacian.T)

    # c1 strips (shared across cores): [T, 128, NT*F1] f16
    cc = np.zeros((T, N, B, K), np.float32)
    cc[..., :H] = states.reshape(T, B, N, H).transpose(0, 2, 1, 3)
    cc[..., H] = inputs.transpose(1, 2, 0)
    c1s = np.ascontiguousarray(
        cc.reshape(T, NT, 128, B * K).transpose(0, 2, 1, 3).reshape(
            T, 128, NT * F1)).astype(ft)

    w1v = np.zeros((K, 2 * H), np.float32)
    w1v[:H] = 0.5 * W1[1:]
    w1v[H] = 0.5 * W1[0]
    w2v = np.zeros((K, H), np.float32)
    w2v[:H] = 0.5 * W2[1:]
    w2v[H] = 0.5 * W2[0]

    bt = np.zeros((T, 2 * B * H), np.float32)
    for t in range(T):
        for half in range(2):
            for b in range(B):
                bt[t, half * B * H + b * H:half * B * H + (b + 1) * H] = \
                    (t + 1.0) * b1[half * H:(half + 1) * H]
    b2v = np.tile(10.0 * b2, B).astype(np.float32)[None, :]

    in_maps = []
    for c in range(NC):
        blk = slice(c * RPC, (c + 1) * RPC)
        mc = np.empty((T, 128, NT * RPC), ft)
        dT = dtwT[:, blk]
        tT = tdcT[:, blk]
        bT = baseT[:, blk]
        for t in range(T):
            MtT = bT + dT * (tT > (9.0 - t))
            if t == T - 1:
                MtT = (MtT + lapT[:, blk]) * (2.0 / 3.0)
            mc[t] = MtT.reshape(NT, 128, RPC).transpose(1, 0, 2).reshape(
                128, NT * RPC).astype(ft)
        in_maps.append(
            dict(mh=mc, c1h=c1s, w1h=w1v, w2h=w2v, biastab=bt, bias2=b2v)
        )
    return in_maps


def kernel(inputs, states, dtw, spec_lap, laplacian, time_delay,
           W1, b1, W2, b2):
    in_maps = make_in_maps(
        np.asarray(inputs, np.float32), np.asarray(states, np.float32),
        np.asarray(dtw, np.float32), np.asarray(spec_lap, np.float32),
        np.asarray(laplacian, np.float32), np.asarray(time_delay, np.float32),
        np.asarray(W1, np.float32), np.asarray(b1, np.float32),
        np.asarray(W2, np.float32), np.asarray(b2, np.float32),
    )
    nc = _get_nc()
    res = run_bass_kernel_spmd(nc, in_maps, CORES, trace=False)
    hv = np.asarray(res.results[0]["hout"], np.float32)   # [128, NT*B*H]
    out = hv.reshape(128, NT, B, H).transpose(2, 1, 0, 3).reshape(B, N * H)
    return np.ascontiguousarray(out)


# revision 10
# speedup vs baseline: 1.0257x; 1.0257x over previous
"""FESTGCN Trainium2 kernel: 8-core SPMD Bass/Tile implementation (v2).

Algorithm (validated against the reference in f32/f16 numpy, sim.py):
  For t in 0..9:
    M_t = dtw * (td > 9-t) + (spec_lap + I)   [(+ lap) * 2/3 at t=9, host-baked]
    S1 += M_t @ c1_t          c1_t = [h_t | x_t]  per-b, k-order [32 h, x, pad]
    gcn1_t = 0.5 * S1 @ W1 + (t+1) b1 ;  sig_t = sigmoid(gcn1_t)
    r gating: gate(node n, ch c) = sig(node n//2, half n%2, ch c)
    S2 += M_t @ c2_t          c2_t = [r*h_t | x_t]
  u(n,c) = sig_9(node N/2 + n//2, half n%2); c = tanh(0.5 * S2 @ W2 + 10 b2)
  out = c + u*(h_9 - c)

Design vs v1 baseline:
  - M_t masks/sums/scaling precomputed on HOST, streamed per step as one
    4MB DMA ([128, 32*512] f16 strip, 32KB/partition descriptors).
  - c1 strips ([x|h] conc) precomputed on HOST in matmul-ready layout:
    one 1MB DMA per step. No on-chip transposes/casts for inputs.
  - AllGather payload pre-EXPANDED to gate-node-major rows (row 2q+h holds
    sig(q, half h)), so gate/u gathers are contiguous 256B-descriptor DMAs.
  - Software pipeline: conv2 of step t-1 runs during step t (AG hidden).
  - Epilogue: tanh-c AllGather (aux), full output mixed redundantly per core.
"""

import numpy as np

import concourse.bacc as bacc
import concourse.mybir as mybir
import concourse.tile as tile
from concourse.bass_utils import run_bass_kernel_spmd

B, T, N, H = 4, 10, 4096, 32
NC = 8
RPC = N // NC            # 512 rows per core
NT = N // 128            # 32 n-tiles
MT = RPC // 128          # 4 m-tiles per core
K = H + 2                # 34: [32 h, x, zero-pad] per-b channel order
F1 = B * K               # 136
NB = NT * B              # 128
f32 = mybir.dt.float32
f16 = mybir.dt.float16
Alu = mybir.AluOpType
Act = mybir.ActivationFunctionType
CORES = list(range(NC))


def _build_nc():
    nc = bacc.Bacc(
        "TRN2",
        target_bir_lowering=False,
        debug=False,
        enable_asserts=True,
        num_devices=NC,
    )
    mh = nc.dram_tensor("mh", [T, 128, NT * RPC], f16, kind="ExternalInput").ap()
    c1h = nc.dram_tensor("c1h", [T, 128, NT * F1], f16, kind="ExternalInput").ap()
    w1h = nc.dram_tensor("w1h", [K, 2 * H], f16, kind="ExternalInput").ap()
    w2h = nc.dram_tensor("w2h", [K, H], f16, kind="ExternalInput").ap()
    biastab = nc.dram_tensor("biastab", [T, 2 * B * H], f32,
                             kind="ExternalInput").ap()
    bias2 = nc.dram_tensor("bias2", [1, B * H], f32, kind="ExternalInput").ap()
    hout = nc.dram_tensor("hout", [128, NT * B * H], f16,
                          kind="ExternalOutput").ap()

    with tile.TileContext(nc) as tc:
        with (
            tc.tile_pool(name="mpool", bufs=3) as mpool,     # M strips (32KB/p)
            tc.tile_pool(name="c1p", bufs=3) as c1p,         # c1 strips
            tc.tile_pool(name="gp", bufs=2) as gp,           # gate/u/cl strips
            tc.tile_pool(name="c2p", bufs=2) as c2p,         # c2 strips
            tc.tile_pool(name="sm", bufs=1) as sm,           # small persistents
            tc.tile_pool(name="acc", bufs=2) as accp,        # S1/S2 ping-pong
            tc.tile_pool(name="wk", bufs=4) as wk,           # small work tiles
            tc.tile_pool(name="z1p", bufs=1, space="PSUM") as z1p,
            tc.tile_pool(name="z2p", bufs=1, space="PSUM") as z2p,
            tc.tile_pool(name="tpz", bufs=1, space="PSUM") as tpzp,
            tc.tile_pool(name="g1p", bufs=2, space="PSUM") as g1p,
            tc.tile_pool(name="dramp", bufs=1, space="DRAM") as dramp,
        ):
            agsrc = [
                dramp.tile([2 * RPC, B * H], f16, tag=f"agsrc{t}",
                           name=f"agsrc{t}")
                for t in range(T)
            ]
            agdst = [
                dramp.tile([NC * 2 * RPC, B * H], f16, tag=f"agdst{t}",
                           name=f"agdst{t}", addr_space="Shared")
                for t in range(T)
            ]
            auxsrc = dramp.tile([RPC, B * H], f16, tag="auxsrc", name="auxsrc")
            auxdst = dramp.tile([NC * RPC, B * H], f16, tag="auxdst",
                                name="auxdst", addr_space="Shared")

            # ---------------- prologue ----------------
            iota_i = wk.tile([128, 128], mybir.dt.int32, tag="iota", bufs=1)
            nc.gpsimd.iota(iota_i[:], pattern=[[1, 128]], base=0,
                           channel_multiplier=-1)
            ident = sm.tile([128, 128], f16, tag="ident")
            nc.vector.tensor_scalar(ident[:], iota_i[:], 0, None,
                                    op0=Alu.is_equal)

            w1s = sm.tile([K, 2 * H], f16, tag="w1s")
            nc.sync.dma_start(w1s[:], w1h[:])
            w2s = sm.tile([K, H], f16, tag="w2s")
            nc.sync.dma_start(w2s[:], w2h[:])
            biasf2 = sm.tile([128, B * H], f32, tag="biasf2")
            nc.sync.dma_start(biasf2[:], bias2[0:1, :].broadcast_to((128, B * H)))

            s1 = [accp.tile([128, F1], f32, tag=f"s1_{mt}", name=f"s1_{mt}")
                  for mt in range(MT)]
            s2 = [accp.tile([128, F1], f32, tag=f"s2_{mt}", name=f"s2_{mt}")
                  for mt in range(MT)]
            for mt in range(MT):
                nc.vector.memset(s1[mt][:], 0.0)
                nc.vector.memset(s2[mt][:], 0.0)

            # step-0 strips
            mtiles = {}
            c1tiles = {}
            mtiles[0] = mpool.tile([128, NT * RPC], f16, tag="m", name="m0")
            nc.sync.dma_start(mtiles[0][:], mh[0])
            c1tiles[0] = c1p.tile([128, NT * F1], f16, tag="c1", name="c10")
            nc.sync.dma_start(c1tiles[0][:], c1h[0])

            def conv(zpool, mstrip, cstrip, ztag):
                za = zpool.tile([128, 2 * F1], f32, tag=f"{ztag}a",
                                name=f"{ztag}a")
                zb = zpool.tile([128, 2 * F1], f32, tag=f"{ztag}b",
                                name=f"{ztag}b")
                zh = [za, zb]
                for mt in range(MT):
                    for nt in range(NT):
                        nc.tensor.matmul(
                            zh[mt // 2][:, (mt % 2) * F1:(mt % 2 + 1) * F1],
                            mstrip[:, nt * RPC + mt * 128:
                                   nt * RPC + (mt + 1) * 128],
                            cstrip[:, nt * F1:(nt + 1) * F1],
                            start=(nt == 0 and mt % 2 == 0),
                            stop=(nt == NT - 1),
                        )
                return zh

            def gates_for(tt):
                gstrip = gp.tile([128, NT * B * H], f16, tag="g",
                                 name=f"g{tt}")
                nc.sync.dma_start(
                    gstrip.rearrange("p (nt c) -> p nt c", c=B * H),
                    agdst[tt][0:N, :].rearrange("(nt p) c -> p nt c", p=128),
                )
                return gstrip

            def conv2_for(tt, gstrip):
                """c2 build + conv2 matmuls + S2 update."""
                c1prev = c1tiles[tt]
                c2 = c2p.tile([128, NT * F1], f16, tag="c2")
                c2v = c2.rearrange("p (nb k) -> p nb k", k=K)
                c1v = c1prev.rearrange("p (nb k) -> p nb k", k=K)
                gv = gstrip.rearrange("p (nb c) -> p nb c", c=H)
                nc.vector.tensor_mul(c2v[:, :, 0:H], gv[:], c1v[:, :, 0:H])
                nc.vector.tensor_copy(c2v[:, :, H:K], c1v[:, :, H:K])
                z2h = conv(z2p, mtiles[tt][:], c2, "z2")
                for mt in range(MT):
                    s2n = accp.tile([128, F1], f32, tag=f"s2_{mt}")
                    nc.vector.tensor_add(
                        s2n[:], s2[mt][:],
                        z2h[mt // 2][:, (mt % 2) * F1:(mt % 2 + 1) * F1],
                    )
                    s2[mt] = s2n

            # ---------------- main loop ----------------
            for t in range(T):
                # gate gather for previous step (scalar ring, waits on AG t-1)
                gcur = gates_for(t - 1) if t >= 1 else None
                # prefetch next step strips
                if t + 1 < T:
                    mtiles[t + 1] = mpool.tile([128, NT * RPC], f16, tag="m",
                                             name=f"m{t + 1}")
                    nc.sync.dma_start(mtiles[t + 1][:], mh[t + 1])
                    c1tiles[t + 1] = c1p.tile([128, NT * F1], f16, tag="c1",
                                              name=f"c1{t + 1}")
                    nc.sync.dma_start(c1tiles[t + 1][:], c1h[t + 1])

                # conv1_t
                z1h = conv(z1p, mtiles[t][:], c1tiles[t][:], "z1")

                # S1 update + small matmuls + sigmoid + AG send
                biasf = wk.tile([128, 2 * B * H], f32, tag="biasf", bufs=2,
                                name="biasf")
                nc.scalar.dma_start(
                    biasf[:], biastab[t:t + 1, :].broadcast_to((128, 2 * B * H)))
                for mt in range(MT):
                    s1n = accp.tile([128, F1], f32, tag=f"s1_{mt}")
                    nc.vector.tensor_add(
                        s1n[:], s1[mt][:],
                        z1h[mt // 2][:, (mt % 2) * F1:(mt % 2 + 1) * F1],
                    )
                    s1[mt] = s1n
                    s1hf = wk.tile([128, F1], f16, tag="s1hf", bufs=2)
                    nc.vector.tensor_copy(s1hf[:], s1n[:])
                    g1 = g1p.tile([128, 2 * B * H], f32, tag="g1")
                    for b in range(B):
                        tz = tpzp.tile([K, 128], f16, tag="tz")
                        nc.tensor.transpose(
                            tz[:], s1hf[:, b * K:(b + 1) * K], ident[:])
                        zbt = wk.tile([K, 128], f16, tag="zbt", bufs=6)
                        nc.scalar.copy(zbt[:], tz[:])
                        nc.tensor.matmul(
                            g1[:, b * H:(b + 1) * H],
                            zbt[:], w1s[:, 0:H], start=True, stop=True)
                        nc.tensor.matmul(
                            g1[:, B * H + b * H:B * H + (b + 1) * H],
                            zbt[:], w1s[:, H:2 * H], start=True, stop=True)
                    sigi = wk.tile([128, 2 * B * H], f32, tag="sigi")
                    nc.vector.scalar_tensor_tensor(
                        sigi[:], g1[:], 1.0, biasf[:],
                        op0=Alu.mult, op1=Alu.add)
                    sigb = wk.tile([128, 2 * B * H], f16, tag="sigb")
                    nc.scalar.activation(sigb[:], sigi[:], Act.Sigmoid)
                    nc.scalar.dma_start(
                        agsrc[t][2 * mt * 128:2 * (mt + 1) * 128, :]
                        .rearrange("(p h) c -> p h c", h=2),
                        sigb.rearrange("p (h c) -> p h c", h=2),
                    )
                nc.gpsimd.collective_compute(
                    "AllGather",
                    Alu.bypass,
                    replica_groups=[CORES],
                    ins=[agsrc[t][:]],
                    outs=[agdst[t][:]],
                )

                # conv2 for previous step (AG t-1 already landed / landing)
                if t >= 1:
                    conv2_for(t - 1, gcur)

            # ---------------- epilogue ----------------
            gfin = gates_for(T - 1)
            conv2_for(T - 1, gfin)

            # u strip prefetch (ready once AG_9 lands; overlaps tanh/aux AG)
            ustrip = gp.tile([128, NT * B * H], f16, tag="g", name="ustrip")
            nc.scalar.dma_start(
                ustrip.rearrange("p (nt c) -> p nt c", c=B * H),
                agdst[T - 1][N:2 * N, :].rearrange("(nt p) c -> p nt c", p=128),
            )

            # tanh path (own rows) -> aux AG
            for mt in range(MT):
                s2hf = wk.tile([128, F1], f16, tag="s1hf", bufs=2, name="s2hf")
                nc.vector.tensor_copy(s2hf[:], s2[mt][:])
                g2full = g1p.tile([128, 2 * B * H], f32, tag="g1", name="g2")
                g2 = g2full[:, 0:B * H]
                for b in range(B):
                    tz = tpzp.tile([K, 128], f16, tag="tz")
                    nc.tensor.transpose(
                        tz[:], s2hf[:, b * K:(b + 1) * K], ident[:])
                    zbt = wk.tile([K, 128], f16, tag="zbt", bufs=6)
                    nc.scalar.copy(zbt[:], tz[:])
                    nc.tensor.matmul(g2[:, b * H:(b + 1) * H], zbt[:], w2s[:],
                                     start=True, stop=True)
                tani = wk.tile([128, B * H], f32, tag="tani", bufs=2)
                nc.vector.scalar_tensor_tensor(
                    tani[:], g2[:, :], 1.0, biasf2[:], op0=Alu.mult, op1=Alu.add)
                tanb = wk.tile([128, B * H], f16, tag="tanb", bufs=2)
                nc.scalar.activation(tanb[:], tani[:], Act.Tanh)
                nc.sync.dma_start(auxsrc[mt * 128:(mt + 1) * 128, :], tanb[:])
            nc.gpsimd.collective_compute(
                "AllGather",
                Alu.bypass,
                replica_groups=[CORES],
                ins=[auxsrc[:]],
                outs=[auxdst[:]],
            )

            # full-output mix (redundant on every core)
            clstrip = gp.tile([128, NT * B * H], f16, tag="g", name="clstrip")
            nc.scalar.dma_start(
                clstrip.rearrange("p (nt c) -> p nt c", c=B * H),
                auxdst[:].rearrange("(nt p) c -> p nt c", p=128),
            )
            c1v9 = c1tiles[T - 1].rearrange("p (nb k) -> p nb k", k=K)
            clv = clstrip.rearrange("p (nb c) -> p nb c", c=H)
            ds = c2p.tile([128, NT * B * H], f16, tag="ds", bufs=2, name="ds")
            nc.vector.tensor_sub(
                ds.rearrange("p (nb c) -> p nb c", c=H),
                c1v9[:, :, 0:H], clv[:])
            ms = c2p.tile([128, NT * B * H], f16, tag="ds", name="ms")
            nc.vector.tensor_mul(ms[:], ustrip[:], ds[:])
            ho = gp.tile([128, NT * B * H], f16, tag="g", name="ho")
            nc.vector.tensor_add(ho[:], ms[:], clstrip[:])
            nc.sync.dma_start(hout[:], ho[:])

    nc.finalize()
    return nc


_NC_CACHE = None


def _get_nc():
    global _NC_CACHE
    if _NC_CACHE is None:
        _NC_CACHE = _build_nc()
    return _NC_CACHE


def make_in_maps(inputs, states, dtw, spec_lap, laplacian, time_delay,
                 W1, b1, W2, b2):
    ft = np.float16
    tdc = np.ceil(np.abs(time_delay) / 1.0).astype(np.float32)
    base = spec_lap + np.eye(N, dtype=np.float32)
    baseT = np.ascontiguousarray(base.T)
    dtwT = np.ascontiguousarray(dtw.T)
    tdcT = np.ascontiguousarray(tdc.T)
    lapT = np.ascontiguousarray(laplacian.T)

    # c1 strips (shared across cores): [T, 128, NT*F1] f16
    cc = np.zeros((T, N, B, K), np.float32)
    cc[..., :H] = states.reshape(T, B, N, H).transpose(0, 2, 1, 3)
    cc[..., H] = inputs.transpose(1, 2, 0)
    c1s = np.ascontiguousarray(
        cc.reshape(T, NT, 128, B * K).transpose(0, 2, 1, 3).reshape(
            T, 128, NT * F1)).astype(ft)

    w1v = np.zeros((K, 2 * H), np.float16)
    w1v[:H] = (0.5 * W1[1:]).astype(np.float16)
    w1v[H] = (0.5 * W1[0]).astype(np.float16)
    w2v = np.zeros((K, H), np.float16)
    w2v[:H] = (0.5 * W2[1:]).astype(np.float16)
    w2v[H] = (0.5 * W2[0]).astype(np.float16)

    bt = np.zeros((T, 2 * B * H), np.float32)
    for t in range(T):
        for half in range(2):
            for b in range(B):
                bt[t, half * B * H + b * H:half * B * H + (b + 1) * H] = \
                    (t + 1.0) * b1[half * H:(half + 1) * H]
    b2v = np.tile(10.0 * b2, B).astype(np.float32)[None, :]

    in_maps = []
    for c in range(NC):
        blk = slice(c * RPC, (c + 1) * RPC)
        mc = np.empty((T, 128, NT * RPC), ft)
        dT = dtwT[:, blk]
        tT = tdcT[:, blk]
        bT = baseT[:, blk]
        for t in range(T):
            MtT = bT + dT * (tT > (9.0 - t))
            if t == T - 1:
                MtT = (MtT + lapT[:, blk]) * (2.0 / 3.0)
            mc[t] = MtT.reshape(NT, 128, RPC).transpose(1, 0, 2).reshape(
                128, NT * RPC).astype(ft)
        in_maps.append(
            dict(mh=mc, c1h=c1s, w1h=w1v, w2h=w2v, biastab=bt, bias2=b2v)
        )
    return in_maps


def kernel(inputs, states, dtw, spec_lap, laplacian, time_delay,
           W1, b1, W2, b2):
    in_maps = make_in_maps(
        np.asarray(inputs, np.float32), np.asarray(states, np.float32),
        np.asarray(dtw, np.float32), np.asarray(spec_lap, np.float32),
        np.asarray(laplacian, np.float32), np.asarray(time_delay, np.float32),
        np.asarray(W1, np.float32), np.asarray(b1, np.float32),
        np.asarray(W2, np.float32), np.asarray(b2, np.float32),
    )
    nc = _get_nc()
    res = run_bass_kernel_spmd(nc, in_maps, CORES, trace=False)
    hv = np.asarray(res.results[0]["hout"], np.float32)   # [128, NT*B*H]
    out = hv.reshape(128, NT, B, H).transpose(2, 1, 0, 3).reshape(B, N * H)
    return np.ascontiguousarray(out)


# revision 11
# speedup vs baseline: 1.0259x; 1.0002x over previous
"""FESTGCN Trainium2 kernel: 8-core SPMD Bass/Tile implementation (v2).

Algorithm (validated against the reference in f32/f16 numpy, sim.py):
  For t in 0..9:
    M_t = dtw * (td > 9-t) + (spec_lap + I)   [(+ lap) * 2/3 at t=9, host-baked]
    S1 += M_t @ c1_t          c1_t = [h_t | x_t]  per-b, k-order [32 h, x, pad]
    gcn1_t = 0.5 * S1 @ W1 + (t+1) b1 ;  sig_t = sigmoid(gcn1_t)
    r gating: gate(node n, ch c) = sig(node n//2, half n%2, ch c)
    S2 += M_t @ c2_t          c2_t = [r*h_t | x_t]
  u(n,c) = sig_9(node N/2 + n//2, half n%2); c = tanh(0.5 * S2 @ W2 + 10 b2)
  out = c + u*(h_9 - c)

Design vs v1 baseline:
  - M_t masks/sums/scaling precomputed on HOST, streamed per step as one
    4MB DMA ([128, 32*512] f16 strip, 32KB/partition descriptors).
  - c1 strips ([x|h] conc) precomputed on HOST in matmul-ready layout:
    one 1MB DMA per step. No on-chip transposes/casts for inputs.
  - AllGather payload pre-EXPANDED to gate-node-major rows (row 2q+h holds
    sig(q, half h)), so gate/u gathers are contiguous 256B-descriptor DMAs.
  - Software pipeline: conv2 of step t-1 runs during step t (AG hidden).
  - Epilogue: tanh-c AllGather (aux), full output mixed redundantly per core.
"""

import numpy as np

import concourse.bacc as bacc
import concourse.mybir as mybir
import concourse.tile as tile
from concourse.bass_utils import run_bass_kernel_spmd

B, T, N, H = 4, 10, 4096, 32
NC = 8
RPC = N // NC            # 512 rows per core
NT = N // 128            # 32 n-tiles
MT = RPC // 128          # 4 m-tiles per core
K = H + 2                # 34: [32 h, x, zero-pad] per-b channel order
F1 = B * K               # 136
NB = NT * B              # 128
f32 = mybir.dt.float32
f16 = mybir.dt.float16
Alu = mybir.AluOpType
Act = mybir.ActivationFunctionType
CORES = list(range(NC))


def _build_nc():
    nc = bacc.Bacc(
        "TRN2",
        target_bir_lowering=False,
        debug=False,
        enable_asserts=True,
        num_devices=NC,
    )
    mh = nc.dram_tensor("mh", [T, 128, NT * RPC], f16, kind="ExternalInput").ap()
    c1h = nc.dram_tensor("c1h", [T, 128, NT * F1], f16, kind="ExternalInput").ap()
    w1h = nc.dram_tensor("w1h", [K, 2 * H], f16, kind="ExternalInput").ap()
    w2h = nc.dram_tensor("w2h", [K, H], f16, kind="ExternalInput").ap()
    biastab = nc.dram_tensor("biastab", [T, 2 * B * H], f32,
                             kind="ExternalInput").ap()
    bias2 = nc.dram_tensor("bias2", [1, B * H], f32, kind="ExternalInput").ap()
    hout = nc.dram_tensor("hout", [128, NT * B * H], f16,
                          kind="ExternalOutput").ap()

    with tile.TileContext(nc) as tc:
        with (
            tc.tile_pool(name="mpool", bufs=3) as mpool,     # M strips (32KB/p)
            tc.tile_pool(name="c1p", bufs=3) as c1p,         # c1 strips
            tc.tile_pool(name="gp", bufs=2) as gp,           # gate/u/cl strips
            tc.tile_pool(name="c2p", bufs=2) as c2p,         # c2 strips
            tc.tile_pool(name="sm", bufs=1) as sm,           # small persistents
            tc.tile_pool(name="acc", bufs=2) as accp,        # S1/S2 ping-pong
            tc.tile_pool(name="wk", bufs=4) as wk,           # small work tiles
            tc.tile_pool(name="z1p", bufs=1, space="PSUM") as z1p,
            tc.tile_pool(name="z2p", bufs=1, space="PSUM") as z2p,
            tc.tile_pool(name="tpz", bufs=1, space="PSUM") as tpzp,
            tc.tile_pool(name="g1p", bufs=2, space="PSUM") as g1p,
            tc.tile_pool(name="dramp", bufs=1, space="DRAM") as dramp,
        ):
            agsrc = [
                dramp.tile([2 * RPC, B * H], f16, tag=f"agsrc{t}",
                           name=f"agsrc{t}")
                for t in range(T)
            ]
            agdst = [
                dramp.tile([NC * 2 * RPC, B * H], f16, tag=f"agdst{t}",
                           name=f"agdst{t}", addr_space="Shared")
                for t in range(T)
            ]
            auxsrc = dramp.tile([RPC, B * H], f16, tag="auxsrc", name="auxsrc")
            auxdst = dramp.tile([NC * RPC, B * H], f16, tag="auxdst",
                                name="auxdst", addr_space="Shared")

            # ---------------- prologue ----------------
            iota_i = wk.tile([128, 128], mybir.dt.int32, tag="iota", bufs=1)
            nc.gpsimd.iota(iota_i[:], pattern=[[1, 128]], base=0,
                           channel_multiplier=-1)
            ident = sm.tile([128, 128], f16, tag="ident")
            nc.vector.tensor_scalar(ident[:], iota_i[:], 0, None,
                                    op0=Alu.is_equal)

            w1s = sm.tile([K, 2 * H], f16, tag="w1s")
            nc.sync.dma_start(w1s[:], w1h[:])
            w2s = sm.tile([K, H], f16, tag="w2s")
            nc.sync.dma_start(w2s[:], w2h[:])
            biasf2 = sm.tile([128, B * H], f32, tag="biasf2")
            nc.sync.dma_start(biasf2[:], bias2[0:1, :].broadcast_to((128, B * H)))

            s1 = [accp.tile([128, F1], f32, tag=f"s1_{mt}", name=f"s1_{mt}")
                  for mt in range(MT)]
            s2 = [accp.tile([128, F1], f32, tag=f"s2_{mt}", name=f"s2_{mt}")
                  for mt in range(MT)]
            for mt in range(MT):
                nc.vector.memset(s1[mt][:], 0.0)
                nc.vector.memset(s2[mt][:], 0.0)

            # step-0 strips
            mtiles = {}
            c1tiles = {}
            mtiles[0] = mpool.tile([128, NT * RPC], f16, tag="m", name="m0")
            nc.sync.dma_start(mtiles[0][:], mh[0])
            c1tiles[0] = c1p.tile([128, NT * F1], f16, tag="c1", name="c10")
            nc.sync.dma_start(c1tiles[0][:], c1h[0])

            def conv(zpool, mstrip, cstrip, ztag):
                za = zpool.tile([128, 2 * F1], f32, tag=f"{ztag}a",
                                name=f"{ztag}a")
                zb = zpool.tile([128, 2 * F1], f32, tag=f"{ztag}b",
                                name=f"{ztag}b")
                zh = [za, zb]
                for mt in range(MT):
                    for nt in range(NT):
                        nc.tensor.matmul(
                            zh[mt // 2][:, (mt % 2) * F1:(mt % 2 + 1) * F1],
                            mstrip[:, nt * RPC + mt * 128:
                                   nt * RPC + (mt + 1) * 128],
                            cstrip[:, nt * F1:(nt + 1) * F1],
                            start=(nt == 0 and mt % 2 == 0),
                            stop=(nt == NT - 1),
                        )
                return zh

            def gates_for(tt):
                gstrip = gp.tile([128, NT * B * H], f16, tag="g",
                                 name=f"g{tt}")
                nc.gpsimd.dma_start(
                    gstrip.rearrange("p (nt c) -> p nt c", c=B * H),
                    agdst[tt][0:N, :].rearrange("(nt p) c -> p nt c", p=128),
                )
                return gstrip

            def conv2_for(tt, gstrip):
                """c2 build + conv2 matmuls + S2 update."""
                c1prev = c1tiles[tt]
                c2 = c2p.tile([128, NT * F1], f16, tag="c2")
                c2v = c2.rearrange("p (nb k) -> p nb k", k=K)
                c1v = c1prev.rearrange("p (nb k) -> p nb k", k=K)
                gv = gstrip.rearrange("p (nb c) -> p nb c", c=H)
                NQ = NB // 4
                for q in range(4):
                    sl = slice(q * NQ, (q + 1) * NQ)
                    nc.vector.tensor_mul(
                        c2v[:, sl, 0:H], gv[:, sl, :], c1v[:, sl, 0:H])
                    nc.vector.tensor_copy(
                        c2v[:, sl, H:K], c1v[:, sl, H:K])
                z2h = conv(z2p, mtiles[tt][:], c2, "z2")
                for mt in range(MT):
                    s2n = accp.tile([128, F1], f32, tag=f"s2_{mt}")
                    nc.vector.tensor_add(
                        s2n[:], s2[mt][:],
                        z2h[mt // 2][:, (mt % 2) * F1:(mt % 2 + 1) * F1],
                    )
                    s2[mt] = s2n

            # ---------------- main loop ----------------
            for t in range(T):
                # gate gather for previous step (scalar ring, waits on AG t-1)
                gcur = gates_for(t - 1) if t >= 1 else None
                # prefetch next step strips
                if t + 1 < T:
                    mtiles[t + 1] = mpool.tile([128, NT * RPC], f16, tag="m",
                                             name=f"m{t + 1}")
                    nc.sync.dma_start(mtiles[t + 1][:], mh[t + 1])
                    c1tiles[t + 1] = c1p.tile([128, NT * F1], f16, tag="c1",
                                              name=f"c1{t + 1}")
                    nc.sync.dma_start(c1tiles[t + 1][:], c1h[t + 1])

                # conv1_t
                z1h = conv(z1p, mtiles[t][:], c1tiles[t][:], "z1")

                # S1 update + small matmuls + sigmoid + AG send
                biasf = wk.tile([128, 2 * B * H], f32, tag="biasf", bufs=2,
                                name="biasf")
                nc.scalar.dma_start(
                    biasf[:], biastab[t:t + 1, :].broadcast_to((128, 2 * B * H)))
                for mt in range(MT):
                    s1n = accp.tile([128, F1], f32, tag=f"s1_{mt}")
                    nc.vector.tensor_add(
                        s1n[:], s1[mt][:],
                        z1h[mt // 2][:, (mt % 2) * F1:(mt % 2 + 1) * F1],
                    )
                    s1[mt] = s1n
                    s1hf = wk.tile([128, F1], f16, tag="s1hf", bufs=2)
                    nc.scalar.copy(s1hf[:], s1n[:])
                    g1 = g1p.tile([128, 2 * B * H], f32, tag="g1")
                    for b in range(B):
                        tz = tpzp.tile([K, 128], f16, tag="tz")
                        nc.tensor.transpose(
                            tz[:], s1hf[:, b * K:(b + 1) * K], ident[:])
                        zbt = wk.tile([K, 128], f16, tag="zbt", bufs=6)
                        nc.scalar.copy(zbt[:], tz[:])
                        nc.tensor.matmul(
                            g1[:, b * H:(b + 1) * H],
                            zbt[:], w1s[:, 0:H], start=True, stop=True)
                        nc.tensor.matmul(
                            g1[:, B * H + b * H:B * H + (b + 1) * H],
                            zbt[:], w1s[:, H:2 * H], start=True, stop=True)
                    sigi = wk.tile([128, 2 * B * H], f32, tag="sigi")
                    nc.vector.scalar_tensor_tensor(
                        sigi[:], g1[:], 1.0, biasf[:],
                        op0=Alu.mult, op1=Alu.add)
                    sigb = wk.tile([128, 2 * B * H], f16, tag="sigb")
                    nc.scalar.activation(sigb[:], sigi[:], Act.Sigmoid)
                    nc.scalar.dma_start(
                        agsrc[t][2 * mt * 128:2 * (mt + 1) * 128, :]
                        .rearrange("(p h) c -> p h c", h=2),
                        sigb.rearrange("p (h c) -> p h c", h=2),
                    )
                nc.gpsimd.collective_compute(
                    "AllGather",
                    Alu.bypass,
                    replica_groups=[CORES],
                    ins=[agsrc[t][:]],
                    outs=[agdst[t][:]],
                )

                # conv2 for previous step (AG t-1 already landed / landing)
                if t >= 1:
                    conv2_for(t - 1, gcur)

            # ---------------- epilogue ----------------
            gfin = gates_for(T - 1)
            conv2_for(T - 1, gfin)

            # u strip prefetch (ready once AG_9 lands; overlaps tanh/aux AG)
            ustrip = gp.tile([128, NT * B * H], f16, tag="g", name="ustrip")
            nc.gpsimd.dma_start(
                ustrip.rearrange("p (nt c) -> p nt c", c=B * H),
                agdst[T - 1][N:2 * N, :].rearrange("(nt p) c -> p nt c", p=128),
            )

            # tanh path (own rows) -> aux AG
            for mt in range(MT):
                s2hf = wk.tile([128, F1], f16, tag="s1hf", bufs=2, name="s2hf")
                nc.scalar.copy(s2hf[:], s2[mt][:])
                g2full = g1p.tile([128, 2 * B * H], f32, tag="g1", name="g2")
                g2 = g2full[:, 0:B * H]
                for b in range(B):
                    tz = tpzp.tile([K, 128], f16, tag="tz")
                    nc.tensor.transpose(
                        tz[:], s2hf[:, b * K:(b + 1) * K], ident[:])
                    zbt = wk.tile([K, 128], f16, tag="zbt", bufs=6)
                    nc.scalar.copy(zbt[:], tz[:])
                    nc.tensor.matmul(g2[:, b * H:(b + 1) * H], zbt[:], w2s[:],
                                     start=True, stop=True)
                tani = wk.tile([128, B * H], f32, tag="tani", bufs=2)
                nc.vector.scalar_tensor_tensor(
                    tani[:], g2[:, :], 1.0, biasf2[:], op0=Alu.mult, op1=Alu.add)
                tanb = wk.tile([128, B * H], f16, tag="tanb", bufs=2)
                nc.scalar.activation(tanb[:], tani[:], Act.Tanh)
                nc.sync.dma_start(auxsrc[mt * 128:(mt + 1) * 128, :], tanb[:])
            nc.gpsimd.collective_compute(
                "AllGather",
                Alu.bypass,
                replica_groups=[CORES],
                ins=[auxsrc[:]],
                outs=[auxdst[:]],
            )

            # full-output mix (redundant on every core)
            clstrip = gp.tile([128, NT * B * H], f16, tag="g", name="clstrip")
            nc.gpsimd.dma_start(
                clstrip.rearrange("p (nt c) -> p nt c", c=B * H),
                auxdst[:].rearrange("(nt p) c -> p nt c", p=128),
            )
            c1v9 = c1tiles[T - 1].rearrange("p (nb k) -> p nb k", k=K)
            clv = clstrip.rearrange("p (nb c) -> p nb c", c=H)
            ds = c2p.tile([128, NT * B * H], f16, tag="ds", bufs=2, name="ds")
            nc.vector.tensor_sub(
                ds.rearrange("p (nb c) -> p nb c", c=H),
                c1v9[:, :, 0:H], clv[:])
            ms = c2p.tile([128, NT * B * H], f16, tag="ds", name="ms")
            nc.vector.tensor_mul(ms[:], ustrip[:], ds[:])
            ho = gp.tile([128, NT * B * H], f16, tag="g", name="ho")
            nc.vector.tensor_add(ho[:], ms[:], clstrip[:])
            nc.sync.dma_start(hout[:], ho[:])

    nc.finalize()
    return nc


_NC_CACHE = None


def _get_nc():
    global _NC_CACHE
    if _NC_CACHE is None:
        _NC_CACHE = _build_nc()
    return _NC_CACHE


def make_in_maps(inputs, states, dtw, spec_lap, laplacian, time_delay,
                 W1, b1, W2, b2):
    ft = np.float16
    tdc = np.ceil(np.abs(time_delay) / 1.0).astype(np.float32)
    base = spec_lap + np.eye(N, dtype=np.float32)
    baseT = np.ascontiguousarray(base.T)
    dtwT = np.ascontiguousarray(dtw.T)
    tdcT = np.ascontiguousarray(tdc.T)
    lapT = np.ascontiguousarray(laplacian.T)

    # c1 strips (shared across cores): [T, 128, NT*F1] f16
    cc = np.zeros((T, N, B, K), np.float32)
    cc[..., :H] = states.reshape(T, B, N, H).transpose(0, 2, 1, 3)
    cc[..., H] = inputs.transpose(1, 2, 0)
    c1s = np.ascontiguousarray(
        cc.reshape(T, NT, 128, B * K).transpose(0, 2, 1, 3).reshape(
            T, 128, NT * F1)).astype(ft)

    w1v = np.zeros((K, 2 * H), np.float16)
    w1v[:H] = (0.5 * W1[1:]).astype(np.float16)
    w1v[H] = (0.5 * W1[0]).astype(np.float16)
    w2v = np.zeros((K, H), np.float16)
    w2v[:H] = (0.5 * W2[1:]).astype(np.float16)
    w2v[H] = (0.5 * W2[0]).astype(np.float16)

    bt = np.zeros((T, 2 * B * H), np.float32)
    for t in range(T):
        for half in range(2):
            for b in range(B):
                bt[t, half * B * H + b * H:half * B * H + (b + 1) * H] = \
                    (t + 1.0) * b1[half * H:(half + 1) * H]
    b2v = np.tile(10.0 * b2, B).astype(np.float32)[None, :]

    in_maps = []
    for c in range(NC):
        blk = slice(c * RPC, (c + 1) * RPC)
        mc = np.empty((T, 128, NT * RPC), ft)
        dT = dtwT[:, blk]
        tT = tdcT[:, blk]
        bT = baseT[:, blk]
        for t in range(T):
            MtT = bT + dT * (tT > (9.0 - t))
            if t == T - 1:
                MtT = (MtT + lapT[:, blk]) * (2.0 / 3.0)
            mc[t] = MtT.reshape(NT, 128, RPC).transpose(1, 0, 2).reshape(
                128, NT * RPC).astype(ft)
        in_maps.append(
            dict(mh=mc, c1h=c1s, w1h=w1v, w2h=w2v, biastab=bt, bias2=b2v)
        )
    return in_maps


def kernel(inputs, states, dtw, spec_lap, laplacian, time_delay,
           W1, b1, W2, b2):
    in_maps = make_in_maps(
        np.asarray(inputs, np.float32), np.asarray(states, np.float32),
        np.asarray(dtw, np.float32), np.asarray(spec_lap, np.float32),
        np.asarray(laplacian, np.float32), np.asarray(time_delay, np.float32),
        np.asarray(W1, np.float32), np.asarray(b1, np.float32),
        np.asarray(W2, np.float32), np.asarray(b2, np.float32),
    )
    nc = _get_nc()
    res = run_bass_kernel_spmd(nc, in_maps, CORES, trace=False)
    hv = np.asarray(res.results[0]["hout"], np.float32)   # [128, NT*B*H]
    out = hv.reshape(128, NT, B, H).transpose(2, 1, 0, 3).reshape(B, N * H)
    return np.ascontiguousarray(out)
